# revision 11
# baseline (speedup 1.0000x reference)
"""Trainium2 Bass kernel for LorentzMultiHeadAttention (GQA + RMSnorm + RoPE + causal).

Sharding: 8 cores = 4 batches x 2 head-groups (8 q-heads / 2 kv-heads per core).
Each core computes a partial output out_g = attn(heads_g) @ Wo[:, cols_g].T for its
batch; host sums the two head-group partials per batch.

All matmuls run as float32r (full fp32 storage; TF32-like PE path, 1 cycle/row at
moving-dim >= 256). Attention uses a transposed-S formulation: S^T[j,i] tiles ->
exp (ACT, scale folded) -> P^T directly in SBUF -> O^T accumulation with V as
stationary; denominator via E-matrix matmuls into a shared PSUM bank; normalize
O^T columns with gpsimd partition-broadcast of 1/den.
"""
import numpy as np

B, T, DM = 4, 2048, 2048
H, HKV, D = 16, 4, 128
NH, NKV = 8, 2          # per-core q heads / kv heads
P = 128
TI = T // P             # 16 t-tiles
KT = DM // P            # 16 contraction tiles
EPS = 1.1920929e-7      # finfo(float32).eps
SCALE = 0.08838834764831845  # 1/sqrt(128)
ROPE_BASE = 10000.0
NEG = -1.0e30


def _build_nc():
    import concourse.bass as bass
    import concourse.mybir as mybir
    import concourse.tile as tile
    from concourse import bacc
    from contextlib import ExitStack

    F32 = mybir.dt.float32
    F32R = mybir.dt.float32r
    AF = mybir.ActivationFunctionType

    def r(ap):
        return ap.bitcast(F32R)

    nc = bacc.Bacc(None, target_bir_lowering=False, debug=False)

    xt_d = nc.declare_dram_parameter("xt", [DM, T], F32R, isOutput=False)
    wq_d = nc.declare_dram_parameter("wq", [DM, NH * D], F32R, isOutput=False)
    wk_d = nc.declare_dram_parameter("wk", [DM, NKV * D], F32R, isOutput=False)
    wv_d = nc.declare_dram_parameter("wv", [DM, NKV * D], F32R, isOutput=False)
    wo_d = nc.declare_dram_parameter("wo", [NH * D, DM], F32R, isOutput=False)
    cs_d = nc.declare_dram_parameter("cs", [P, TI * 64], F32, isOutput=False)
    sn_d = nc.declare_dram_parameter("sn", [P, TI * 64], F32, isOutput=False)
    dmask_d = nc.declare_dram_parameter("dmask", [P, P], F32, isOutput=False)
    ident_d = nc.declare_dram_parameter("ident", [P, P], F32, isOutput=False)
    e4_d = nc.declare_dram_parameter("e4", [P, 512], F32R, isOutput=False)
    out_d = nc.declare_dram_parameter("out", [T, DM], F32, isOutput=True)


    with tile.TileContext(nc) as tc:
        with ExitStack() as outer:
            constp = outer.enter_context(tc.tile_pool(name="const", bufs=1))
            ident = constp.tile([P, P], F32)
            dmask = constp.tile([P, P], F32)
            e4 = constp.tile([P, 512], F32R)
            nc.sync.dma_start(out=ident[:], in_=ident_d[:])
            nc.sync.dma_start(out=dmask[:], in_=dmask_d[:])
            nc.sync.dma_start(out=e4[:], in_=e4_d[:])
            eps_t = constp.tile([P, 1], F32)
            nc.vector.memset(eps_t[:], EPS)
            # persistent across phases
            kT = constp.tile([P, NKV, T], F32R)          # [d, hk, t]
            vA = constp.tile([P, TI, NKV, D], F32R)      # [j-in-tile, ti, hk, d]
            dramp = outer.enter_context(tc.tile_pool(name="dram", bufs=1, space="DRAM"))
            qt = dramp.tile([NH, D, T], F32R)            # q^T spill per head

            # ---------------- Phase 1: projections + norm + rope + transpose
            with ExitStack() as ph1:
                wp = ph1.enter_context(tc.tile_pool(name="wp", bufs=1))
                wq = wp.tile([P, KT, NH * D], F32R)
                wk = wp.tile([P, KT, NKV * D], F32R)
                wv = wp.tile([P, KT, NKV * D], F32R)
                cs = wp.tile([P, TI * 64], F32)
                sn = wp.tile([P, TI * 64], F32)
                nc.sync.dma_start(out=wq[:], in_=wq_d[:].rearrange("(kt p) c -> p kt c", p=P))
                nc.sync.dma_start(out=wk[:], in_=wk_d[:].rearrange("(kt p) c -> p kt c", p=P))
                nc.sync.dma_start(out=wv[:], in_=wv_d[:].rearrange("(kt p) c -> p kt c", p=P))
                nc.sync.dma_start(out=cs[:], in_=cs_d[:])
                nc.sync.dma_start(out=sn[:], in_=sn_d[:])

                xtp = ph1.enter_context(tc.tile_pool(name="xtp", bufs=2))
                stg = ph1.enter_context(tc.tile_pool(name="stg", bufs=3))
                sml = ph1.enter_context(tc.tile_pool(name="sml", bufs=4))
                psq = ph1.enter_context(tc.tile_pool(name="psq", bufs=2, space="PSUM"))
                pst = ph1.enter_context(tc.tile_pool(name="pst", bufs=2, space="PSUM"))

                for ti in range(TI):
                    xt_t = xtp.tile([P, KT, P], F32R, tag="xt")
                    nc.sync.dma_start(
                        out=xt_t[:],
                        in_=xt_d[:, ti * P:(ti + 1) * P].rearrange("(kt p) t -> p kt t", p=P),
                    )
                    q0 = psq.tile([P, 512], F32, tag="q0")
                    q1 = psq.tile([P, 512], F32, tag="q1")
                    kv = psq.tile([P, 512], F32, tag="kv")
                    for kt in range(KT):
                        st, sp = kt == 0, kt == KT - 1
                        lhs = xt_t[:, kt, :]
                        nc.tensor.matmul(q0[:], lhs, wq[:, kt, 0:512], start=st, stop=sp)
                        nc.tensor.matmul(q1[:], lhs, wq[:, kt, 512:1024], start=st, stop=sp)
                        nc.tensor.matmul(kv[:, 0:256], lhs, wk[:, kt, :], start=st, stop=sp)
                    for kt in range(KT):
                        st, sp = kt == 0, kt == KT - 1
                        nc.tensor.matmul(kv[:, 256:512], xt_t[:, kt, :], wv[:, kt, :],
                                         start=st, stop=sp, skip_group_check=True)
                    # v blocks straight to SBUF (no norm/rope)
                    for hk in range(NKV):
                        nc.scalar.copy(vA[:, ti, hk, :], kv[:, 256 + 128 * hk:384 + 128 * hk])
                    c_t = cs[:, ti * 64:(ti + 1) * 64]
                    s_t = sn[:, ti * 64:(ti + 1) * 64]
                    for idx in range(NH + NKV):
                        if idx < 4:
                            blk = q0[:, idx * 128:(idx + 1) * 128]
                        elif idx < 8:
                            blk = q1[:, (idx - 4) * 128:(idx - 3) * 128]
                        else:
                            blk = kv[:, (idx - 8) * 128:(idx - 7) * 128]
                        junk = stg.tile([P, P], F32, tag="junk")
                        ss = sml.tile([P, 1], F32, tag="ss")
                        nc.scalar.activation(junk[:], blk, AF.Square, accum_out=ss[:])
                        sq = sml.tile([P, 1], F32, tag="sq")
                        nc.scalar.activation(sq[:], ss[:], AF.Sqrt, scale=1.0 / D, bias=eps_t[:])
                        rr = sml.tile([P, 1], F32, tag="rr")
                        nc.vector.reciprocal(rr[:], sq[:])
                        qn = stg.tile([P, P], F32, tag="qn")
                        nc.vector.tensor_scalar_mul(qn[:], blk, rr[:])
                        ro = stg.tile([P, P], F32, tag="ro")
                        t1 = stg.tile([P, 64], F32, tag="t1")
                        t2 = stg.tile([P, 64], F32, tag="t2")
                        nc.vector.tensor_mul(t1[:], qn[:, 0:64], c_t)
                        nc.vector.tensor_mul(t2[:], qn[:, 64:128], s_t)
                        nc.vector.tensor_add(ro[:, 0:64], t1[:], t2[:])
                        nc.vector.tensor_mul(t1[:], qn[:, 64:128], c_t)
                        nc.vector.tensor_mul(t2[:], qn[:, 0:64], s_t)
                        nc.vector.tensor_sub(ro[:, 64:128], t1[:], t2[:])
                        tp = pst.tile([P, P], F32, tag="tp")
                        nc.tensor.transpose(tp[:], ro[:], ident[:])
                        if idx < NH:
                            so = stg.tile([P, P], F32R, tag="so")
                            nc.scalar.copy(so[:], tp[:])
                            nc.sync.dma_start(out=qt[idx, :, ti * P:(ti + 1) * P], in_=so[:])
                        else:
                            nc.scalar.copy(kT[:, idx - NH, ti * P:(ti + 1) * P], tp[:])

            # ---------------- Phase 2: attention + out-projection
            with ExitStack() as ph2:
                wop = ph2.enter_context(tc.tile_pool(name="wop", bufs=1))
                wo = wop.tile([P, NH, DM], F32R)
                nc.sync.dma_start(out=wo[:], in_=wo_d[:].rearrange("(h p) m -> p h m", p=P))

                qtp = ph2.enter_context(tc.tile_pool(name="qtp", bufs=2))
                ptp = ph2.enter_context(tc.tile_pool(name="ptp", bufs=3))
                onp = ph2.enter_context(tc.tile_pool(name="onp", bufs=9))
                bcp = ph2.enter_context(tc.tile_pool(name="bcp", bufs=2))
                rdp = ph2.enter_context(tc.tile_pool(name="rdp", bufs=2))
                ost = ph2.enter_context(tc.tile_pool(name="ost", bufs=3))
                stps = ph2.enter_context(tc.tile_pool(name="stps", bufs=2, space="PSUM"))
                otps = ph2.enter_context(tc.tile_pool(name="otps", bufs=4, space="PSUM"))
                dnps = ph2.enter_context(tc.tile_pool(name="dnps", bufs=2, space="PSUM"))

                for c in range(4):
                    qt_c = qtp.tile([P, NH, 512], F32R, tag="qt")
                    nc.sync.dma_start(
                        out=qt_c[:],
                        in_=qt[:, :, c * 512:(c + 1) * 512].rearrange("h d t -> d h t"),
                    )
                    njt = 4 * (c + 1)
                    onrm = [None] * NH
                    for hk in range(NKV):
                        ots = [otps.tile([P, 512], F32, tag="ot", name=f"ot{c}_{hk}_{i4}") for i4 in range(4)]
                        den = dnps.tile([P, 512], F32, tag="den")
                        for jt in range(njt):
                            for h4 in range(4):
                                h = hk * 4 + h4
                                stp = stps.tile([P, 512], F32, tag="st")
                                nc.tensor.matmul(
                                    stp[:], kT[:, hk, jt * P:(jt + 1) * P],
                                    qt_c[:, h, :], start=True, stop=True,
                                )
                                off = (jt - 4 * c) * 128
                                if jt >= 4 * c:
                                    nc.vector.tensor_add(
                                        stp[:, off:off + 128], stp[:, off:off + 128], dmask[:]
                                    )
                                    if off > 0:
                                        nc.vector.tensor_scalar_add(
                                            stp[:, 0:off], stp[:, 0:off], NEG
                                        )
                                pt = ptp.tile([P, 512], F32R, tag="pt")
                                nc.scalar.activation(pt[:], stp[:], AF.Exp, scale=SCALE)
                                nc.tensor.matmul(
                                    ots[h4][:], vA[:, jt, hk, :], pt[:],
                                    start=(jt == 0), stop=(jt == njt - 1),
                                    skip_group_check=True,
                                )
                                nc.tensor.matmul(
                                    den[:], e4[:, h4 * 128:(h4 + 1) * 128], pt[:],
                                    start=(jt == 0 and h4 == 0),
                                    stop=(jt == njt - 1 and h4 == 3),
                                    skip_group_check=True,
                                )
                        for h4 in range(4):
                            rd = rdp.tile([1, 512], F32, tag="rd")
                            nc.vector.reciprocal(rd[:], den[32 * h4:32 * h4 + 1, :])
                            bc = bcp.tile([P, 512], F32, tag="bc")
                            nc.gpsimd.partition_broadcast(bc[:], rd[:])
                            on = onp.tile([P, 512], F32R, tag="on")
                            nc.vector.tensor_mul(on[:], ots[h4][:], bc[:])
                            onrm[hk * 4 + h4] = on
                    for tt in range(4):
                        for ms in range(4):
                            op = otps.tile([P, 512], F32, tag="ot")
                            for h in range(NH):
                                nc.tensor.matmul(
                                    op[:], onrm[h][:, tt * 128:(tt + 1) * 128],
                                    wo[:, h, ms * 512:(ms + 1) * 512],
                                    start=(h == 0), stop=(h == NH - 1),
                                    skip_group_check=True,
                                )
                            og = ost.tile([P, 512], F32, tag="og")
                            nc.scalar.copy(og[:], op[:])
                            nc.sync.dma_start(
                                out=out_d[(c * 4 + tt) * P:(c * 4 + tt + 1) * P,
                                          ms * 512:(ms + 1) * 512],
                                in_=og[:],
                            )
    nc.compile()
    return nc


def _host_prep(x, Wq, Wk, Wv, Wo):
    """Per-core input maps: layout-only transforms (transpose/slice/tables)."""
    x = np.ascontiguousarray(np.asarray(x, dtype=np.float32))
    Wq = np.asarray(Wq, dtype=np.float32)
    Wk = np.asarray(Wk, dtype=np.float32)
    Wv = np.asarray(Wv, dtype=np.float32)
    Wo = np.asarray(Wo, dtype=np.float32)

    inv_freq = 1.0 / (ROPE_BASE ** (np.arange(0, D, 2, dtype=np.float32) / D))
    freqs = np.outer(np.arange(T, dtype=np.float32), inv_freq)  # [T, 64]
    cs = np.cos(freqs).astype(np.float32)
    sn = np.sin(freqs).astype(np.float32)
    # [t, i] -> [p, ti*64+i]
    cs_r = np.ascontiguousarray(cs.reshape(TI, P, 64).transpose(1, 0, 2).reshape(P, TI * 64))
    sn_r = np.ascontiguousarray(sn.reshape(TI, P, 64).transpose(1, 0, 2).reshape(P, TI * 64))

    pp = np.arange(P)
    dmask = np.where(pp[None, :] >= pp[:, None], 0.0, NEG).astype(np.float32)  # [j,i] keep i>=j
    ident = np.eye(P, dtype=np.float32)
    e4 = np.zeros((P, 512), dtype=np.float32)
    for h4 in range(4):
        e4[:, h4 * 128 + 32 * h4] = 1.0

    in_maps = []
    for b in range(B):
        xt = np.ascontiguousarray(x[b].T)  # [DM, T]
        for g in range(2):
            qs = slice(g * NH * D, (g + 1) * NH * D)
            ks = slice(g * NKV * D, (g + 1) * NKV * D)
            in_maps.append(dict(
                xt=xt,
                wq=np.ascontiguousarray(Wq[qs, :].T),       # [DM, 1024]
                wk=np.ascontiguousarray(Wk[ks, :].T),       # [DM, 256]
                wv=np.ascontiguousarray(Wv[ks, :].T),       # [DM, 256]
                wo=np.ascontiguousarray(Wo[:, qs].T),       # [1024, DM]
                cs=cs_r, sn=sn_r, dmask=dmask, ident=ident, e4=e4,
            ))
    return in_maps


_NC_CACHE = {}


def run(x, Wq, Wk, Wv, Wo, trace=False):
    from concourse.bass_utils import run_bass_kernel_spmd
    if "nc" not in _NC_CACHE:
        _NC_CACHE["nc"] = _build_nc()
    nc = _NC_CACHE["nc"]
    in_maps = _host_prep(x, Wq, Wk, Wv, Wo)
    res = run_bass_kernel_spmd(nc, in_maps, list(range(8)), trace=trace)
    out = np.zeros((B, T, DM), dtype=np.float32)
    for b in range(B):
        out[b] = res.results[2 * b]["out"] + res.results[2 * b + 1]["out"]
    return out, res


def kernel(x, Wq, Wk, Wv, Wo, temp=None, curvature=None, **_):
    out, _res = run(x, Wq, Wk, Wv, Wo, trace=False)
    return out, np.zeros((1,), dtype=np.float32)


if __name__ == "__main__":
    rng = np.random.default_rng(0)
    x = rng.standard_normal((B, T, DM), dtype=np.float32)
    s = 1.0 / np.sqrt(DM)
    Wq = rng.standard_normal((H * D, DM), dtype=np.float32) * s
    Wk = rng.standard_normal((HKV * D, DM), dtype=np.float32) * s
    Wv = rng.standard_normal((HKV * D, DM), dtype=np.float32) * s
    Wo = rng.standard_normal((DM, H * D), dtype=np.float32) / np.sqrt(H * D)
    out, _ = run(x, Wq, Wk, Wv, Wo)
    print("ran ok", out.shape, float(np.abs(out).max()))
